# revision 15
# baseline (speedup 1.0000x reference)
"""Multi-head causal attention (B=2, T=2048, E=768, H=12, D=64) on 8 trn2 cores.

Sharding: core c handles batch b=c//4 and heads [3g, 3g+1, 3g+2] (g=c%4).
Each core computes its 3 heads' attention plus their partial contribution to the
final projection; the host sums the 4 partials per batch.

Per-core device program (all matmuls bf16 inputs, fp32 PSUM accumulation):
  phase 1: qT/kT = Wqk^T x (+bias via DVE on the PSUM->SBUF move), laid out
           [q0q1][k0k1][q2|k2] over partitions, bf16 in SBUF.
           v computed directly in [token, d] layout: lhsT = xT tile
           (stationary), rhs = wv chunk; bias via a K=1 ones-row matmul.
           v_all [128, NT, 3, 65] holds [v | 1] per head (ones col set once).
  phase 2: per head h, query quarter q (512 cols): for key blocks j in pairs:
           S^T pair -> one merged PSUM tile [128,2,512]; exp on ACT (merged
           over the pair for full blocks); causal masking applied AFTER exp by
           multiplying the diagonal 128-strips of P with a 0/1 lower-triangle
           (one all-SBUF bf16 DVE op per pair); [O^T; l] accumulated via matmul
           with lhsT=[v_j | 1].  Normalize: DVE reciprocal of l, GPSIMD
           partition_broadcast, DVE multiply -> ot in bf16 SBUF.
  phase 3: out = sum_h O_h^T.T @ wf_h -> [2048, 768] fp32 partial, DMA out.
           Interleaved per query quarter to spread PE work and output DMA.

`repeat` unrolls the whole body N times in one NEFF; used by test.py to
measure per-body HW time as a slope over repeat counts.
"""
import numpy as np

EMBED_DIM = 768
B = 2
T = 2048
N_CORES = 8
NT = T // 128           # 16 token tiles
SCALE = 1.0 / np.sqrt(64.0)

_state = {}


def _build(repeat=1):
    import concourse.tile as tile
    from concourse import bacc, mybir

    F32 = mybir.dt.float32
    BF16 = mybir.dt.bfloat16

    nc = bacc.Bacc("TRN2", target_bir_lowering=False, debug=False)

    xT_d = nc.dram_tensor("xT", [EMBED_DIM, T], BF16, kind="ExternalInput").ap()
    # columns ordered [q0 q1 | k0 k1 | q2 | k2]
    wqk_d = nc.dram_tensor("wqk", [EMBED_DIM, 384], BF16, kind="ExternalInput").ap()
    wv_d = nc.dram_tensor("wv", [EMBED_DIM, 192], BF16, kind="ExternalInput").ap()
    bqk_d = nc.dram_tensor("bqk", [384, 1], F32, kind="ExternalInput").ap()
    bvr_d = nc.dram_tensor("bvr", [1, 192], BF16, kind="ExternalInput").ap()
    wf01_d = nc.dram_tensor("wf01", [128, EMBED_DIM], BF16, kind="ExternalInput").ap()
    wf2_d = nc.dram_tensor("wf2", [64, EMBED_DIM], BF16, kind="ExternalInput").ap()
    tri_d = nc.dram_tensor("tri", [128, 256], BF16, kind="ExternalInput").ap()
    import os
    out_dt = BF16 if os.environ.get("KOUT", "f32") == "bf16" else F32
    out_d = nc.dram_tensor("out_p", [T, EMBED_DIM], out_dt, kind="ExternalOutput").ap()

    with tile.TileContext(nc) as tc:
        with tc.tile_pool(name="const", bufs=1) as const, \
             tc.tile_pool(name="persist", bufs=1) as persist:
            # ---- constants ----
            wqk_sb = const.tile([128, 6, 384], BF16)
            wv_sb = const.tile([128, 6, 192], BF16)
            nc.sync.dma_start(out=wqk_sb[:], in_=wqk_d.rearrange("(k p) c -> p k c", p=128))
            nc.sync.dma_start(out=wv_sb[:], in_=wv_d.rearrange("(k p) c -> p k c", p=128))
            bqk_sb = [const.tile([128, 1], F32, name=f"bqk{m}", tag=f"bqk{m}")
                      for m in range(3)]
            for m in range(3):
                nc.sync.dma_start(out=bqk_sb[m][:], in_=bqk_d[128 * m:128 * (m + 1), :])
            bvr_sb = const.tile([1, 192], BF16)
            nc.sync.dma_start(out=bvr_sb[:], in_=bvr_d[:])
            wf01_sb = const.tile([128, EMBED_DIM], BF16)
            wf2_sb = const.tile([64, EMBED_DIM], BF16)
            nc.sync.dma_start(out=wf01_sb[:], in_=wf01_d[:])
            nc.sync.dma_start(out=wf2_sb[:], in_=wf2_d[:])
            tri2_sb = const.tile([128, 2, 128], BF16)
            nc.sync.dma_start(out=tri2_sb[:], in_=tri_d.rearrange("p (a c) -> p a c", a=2))
            ones_row = const.tile([1, 128], BF16)
            nc.vector.memset(ones_row[:], 1.0)

            # ---- persistent activations ----
            qA = persist.tile([128, T], BF16)   # q0 @0:64, q1 @64:128
            kA = persist.tile([128, T], BF16)   # k0 @0:64, k1 @64:128
            qB = persist.tile([64, T], BF16)    # q2
            kB = persist.tile([64, T], BF16)    # k2
            v_all = persist.tile([128, NT, 3, 65], BF16)   # [v | 1] per head
            nc.vector.memset(v_all[:, :, :, 64:65], 1.0)
            ot01 = persist.tile([128, T], BF16)  # normalized O^T heads 0/1
            ot2 = persist.tile([64, T], BF16)    # head 2

            for rep in range(repeat):
                _emit_body(nc, tc, rep, locals())

    nc.compile()
    return nc


def _emit_body(nc, tc, rep, env):
    """Emit one full forward pass.

    PSUM (8 banks): ps0 ps1 ([128,512] phase-1/v/phase-3 rotation),
    stA stB ([128,2,512] S^T pair tiles), otl0 otl1 ([128,512] O^T+l)."""
    from concourse import mybir

    F32 = mybir.dt.float32
    BF16 = mybir.dt.bfloat16
    Exp = mybir.ActivationFunctionType.Exp
    MULT = mybir.AluOpType.mult

    xT_d, out_d = env["xT_d"], env["out_d"]
    wqk_sb, wv_sb = env["wqk_sb"], env["wv_sb"]
    bqk_sb, bvr_sb = env["bqk_sb"], env["bvr_sb"]
    wf01_sb, wf2_sb = env["wf01_sb"], env["wf2_sb"]
    tri2_sb, ones_row = env["tri2_sb"], env["ones_row"]
    qA, kA, qB, kB = env["qA"], env["kA"], env["qB"], env["kB"]
    v_all = env["v_all"]
    ot01, ot2 = env["ot01"], env["ot2"]

    with tc.tile_pool(name=f"sb{rep}", bufs=1) as sbp, \
         tc.tile_pool(name=f"ps{rep}", bufs=1, space="PSUM") as psp:
        # ---- input DMA: one chunk per query quarter, kicked from SP ----
        xT_sb = sbp.tile([128, 6, T], BF16, name=f"xT{rep}", tag=f"xT{rep % 2}")
        xT_r = xT_d.rearrange("(k p) c -> p k c", p=128)
        for n in range(4):
            nsl = slice(512 * n, 512 * (n + 1))
            nc.sync.dma_start(out=xT_sb[:, :, nsl], in_=xT_r[:, :, nsl])

        pidx = [0]

        def qk_group(m, n):
            # m: 0=[q0q1]->qA, 1=[k0k1]->kA, 2=[q2|k2]->qB(0:64)+kB(64:128)
            c0, c1 = 128 * m, 128 * (m + 1)
            ps = psp.tile([128, 512], F32, name=f"pg{rep}_{m}{n}",
                          tag=f"ps{pidx[0] % 2}")
            pidx[0] += 1
            nsl = slice(512 * n, 512 * (n + 1))
            for k in range(6):
                nc.tensor.matmul(ps[:], lhsT=wqk_sb[:, k, c0:c1],
                                 rhs=xT_sb[:, k, nsl], start=(k == 0), stop=(k == 5))
            with nc.allow_low_precision(reason="bf16 activations"):
                if m < 2:
                    dst = qA if m == 0 else kA
                    nc.vector.tensor_scalar_add(out=dst[:, nsl], in0=ps[:],
                                                scalar1=bqk_sb[m][:])
                else:
                    nc.vector.tensor_scalar_add(out=qB[:, nsl], in0=ps[0:64, :],
                                                scalar1=bqk_sb[2][0:64, :])
                    nc.vector.tensor_scalar_add(out=kB[:, nsl], in0=ps[64:128, :],
                                                scalar1=bqk_sb[2][64:128, :])

        def v_tile(i):
            # v in [token, d] layout for the 3 heads + bias via K=1 matmul
            ps = psp.tile([128, 192], F32, name=f"pv{rep}_{i}",
                          tag=f"ps{pidx[0] % 2}")
            pidx[0] += 1
            tsl = slice(128 * i, 128 * (i + 1))
            for k in range(6):
                nc.tensor.matmul(ps[:], lhsT=xT_sb[:, k, tsl],
                                 rhs=wv_sb[:, k, :], start=(k == 0), stop=False)
            nc.tensor.matmul(ps[:], lhsT=ones_row[:], rhs=bvr_sb[:],
                             start=False, stop=True)
            with nc.allow_low_precision(reason="bf16 activations"):
                nc.vector.tensor_copy(out=v_all[:, i, :, 0:64], in_=ps[:])

        head_cfg = [(qA, kA, 0), (qA, kA, 64), (qB, kB, 0)]
        sidx = [0]
        ptidx = [0]
        rbidx = [0]
        otlidx = [0]

        def attn_quarter(h, q):
            """Attention for head h, queries [512q, 512(q+1))."""
            qT, kT, o = head_cfg[h]
            base = 512 * q
            ngroups = 2 * q + 2          # key-block pairs (2 items each)
            otl = psp.tile([128, 512], F32, name=f"otl{rep}_{h}{q}",
                           tag=f"otl{otlidx[0] % 2}")
            otlidx[0] += 1
            groups = []
            for g in range(ngroups):
                items = []
                for jj in range(2):
                    j = 2 * g + jj
                    s0 = max(base, 128 * j)
                    items.append((j, s0, base + 512 - s0))
                st = psp.tile([128, 2, 512], F32, name=f"st{rep}_{h}{q}{g}",
                              tag=f"st{sidx[0] % 2}")
                pt = sbp.tile([128, 2, 512], BF16, name=f"pt{rep}_{h}{q}{g}",
                              tag=f"pt{ptidx[0] % 4}")
                sidx[0] += 1
                ptidx[0] += 1
                groups.append((g, items, st, pt))

            def emit_s(g, items, st, pt):
                for jj, (j, s0, ln) in enumerate(items):
                    nc.tensor.matmul(
                        st[:, jj, 0:ln],
                        lhsT=kT[o:o + 64, 128 * j:128 * (j + 1)],
                        rhs=qT[o:o + 64, s0:s0 + ln],
                        start=True, stop=True)

            def emit_exp(g, items, st, pt):
                diag = items[0][1] == 128 * items[0][0]  # s0 == 128j
                with nc.allow_low_precision(reason="bf16 P"):
                    if not diag:
                        nc.scalar.activation(out=pt[:, :, :], in_=st[:, :, :],
                                             func=Exp, scale=float(SCALE))
                    else:
                        for jj, (j, s0, ln) in enumerate(items):
                            nc.scalar.activation(
                                out=pt[:, jj, 0:ln], in_=st[:, jj, 0:ln],
                                func=Exp, scale=float(SCALE))
                        # zero the masked upper triangle of the diagonal
                        # strips (one all-SBUF bf16 DVE op for both items)
                        nc.vector.tensor_tensor(
                            out=pt[:, :, 0:128], in0=pt[:, :, 0:128],
                            in1=tri2_sb[:], op=MULT)

            def emit_pv(g, items, st, pt):
                for jj, (j, s0, ln) in enumerate(items):
                    nc.tensor.matmul(
                        otl[0:65, s0 - base:512],
                        lhsT=v_all[:, j, h, :], rhs=pt[:, jj, 0:ln],
                        start=(j == 0), stop=(j == 2 * ngroups - 1))

            # software-pipelined emission: S g+1 runs on PE while exp g is on
            # ACT; PV g follows its exp.
            emit_s(*groups[0])
            if ngroups > 1:
                emit_s(*groups[1])
            emit_exp(*groups[0])
            for g in range(ngroups):
                if g + 2 < ngroups:
                    emit_s(*groups[g + 2])
                if g + 1 < ngroups:
                    emit_exp(*groups[g + 1])
                emit_pv(*groups[g])

            # normalize: O^T rows / l row (broadcast the reciprocal over
            # partitions, either on GPSIMD or via a PE ones-matmul)
            import os
            knorm = os.environ.get("KNORM", "pool")
            rsb = sbp.tile([1, 512], F32, name=f"rs{rep}_{h}{q}",
                           tag=f"rs{rbidx[0] % 2}")
            bcast = sbp.tile([64, 512], F32, name=f"bc{rep}_{h}{q}",
                             tag=f"bc{rbidx[0] % 2}")
            rbidx[0] += 1
            ot_dst = ot2 if h == 2 else ot01
            ob_ = 64 if h == 1 else 0
            with nc.allow_low_precision(reason="recip + bf16 O"):
                if knorm == "pool":
                    nc.vector.reciprocal(out=rsb[:], in_=otl[64:65, :])
                    nc.gpsimd.partition_broadcast(bcast[:], rsb[:])
                else:
                    rsb_h = sbp.tile([1, 512], BF16, name=f"rh{rep}_{h}{q}",
                                     tag=f"rh{(rbidx[0] + 1) % 2}")
                    nc.vector.reciprocal(out=rsb_h[:], in_=otl[64:65, :])
                    bcp = psp.tile([64, 512], F32, name=f"bcp{rep}_{h}{q}",
                                   tag=f"ps{pidx[0] % 2}")
                    pidx[0] += 1
                    nc.tensor.matmul(bcp[:], lhsT=ones_row[0:1, 0:64],
                                     rhs=rsb_h[:], start=True, stop=True)
                    nc.vector.tensor_copy(out=bcast[:], in_=bcp[:])
                nc.vector.tensor_tensor(
                    out=ot_dst[ob_:ob_ + 64, base:base + 512],
                    in0=otl[0:64, :], in1=bcast[:], op=MULT)

        def phase3_tile(i):
            a = pidx[0] % 2
            fpa = psp.tile([128, 512], F32, name=f"fpa{rep}_{i}",
                           tag=f"ps{a}")
            fpb = psp.tile([128, 256], F32, name=f"fpb{rep}_{i}",
                           tag=f"ps{1 - a}")
            pidx[0] += 1
            tsl = slice(128 * i, 128 * (i + 1))
            # consecutive matmuls share lhsT (one LDWEIGHTS per operand)
            for (fp, n0, n1) in [(fpa, 0, 512), (fpb, 512, 768)]:
                nc.tensor.matmul(fp[:, 0:n1 - n0], lhsT=ot01[:, tsl],
                                 rhs=wf01_sb[:, n0:n1], start=True, stop=False)
            for (fp, n0, n1) in [(fpa, 0, 512), (fpb, 512, 768)]:
                nc.tensor.matmul(fp[:, 0:n1 - n0], lhsT=ot2[:, tsl],
                                 rhs=wf2_sb[:, n0:n1], start=False, stop=True)
            ob = sbp.tile([128, EMBED_DIM], out_d.dtype, name=f"ob{rep}_{i}",
                          tag=f"ob{i % 3}")
            with nc.allow_low_precision(reason="partial out"):
                nc.vector.tensor_copy(out=ob[:, 0:512], in_=fpa[:])
            nc.vector.tensor_copy(out=ob[:, 512:768], in_=fpb[:])
            nc.sync.dma_start(out=out_d[tsl, :], in_=ob[:])

        # ---- staged interleaved emission ----
        # KPHASES env: "1" = projections only, "12" = + attention, default all
        import os
        kp = os.environ.get("KPHASES", "123")
        if "2" not in kp:
            def attn_quarter(h, q):
                pass
        if "3" not in kp:
            def phase3_tile(i):
                pass

        for n in (0, 1):
            qk_group(0, n)
            qk_group(1, n)
            qk_group(2, n)
            for i in range(4 * n, 4 * n + 4):
                v_tile(i)
        for h in range(3):
            attn_quarter(h, 0)
        qk_group(0, 2)
        qk_group(1, 2)
        qk_group(2, 2)
        for i in range(8, 12):
            v_tile(i)
        phase3_tile(0)
        phase3_tile(1)
        for h in range(3):
            attn_quarter(h, 1)
        qk_group(0, 3)
        qk_group(1, 3)
        qk_group(2, 3)
        for i in range(12, 16):
            v_tile(i)
        phase3_tile(2)
        phase3_tile(3)
        for h in range(3):
            attn_quarter(h, 2)
        for i in range(4, 8):
            phase3_tile(i)
        for h in range(3):
            attn_quarter(h, 3)
        for i in range(8, 16):
            phase3_tile(i)


def _prep_inputs(x, w_qkv, b_qkv, w_final):
    """Build the 8 per-core input maps from the full inputs."""
    import ml_dtypes
    bf16 = ml_dtypes.bfloat16

    x = np.asarray(x, dtype=np.float32)
    w_qkv = np.asarray(w_qkv, dtype=np.float32)
    b_qkv = np.asarray(b_qkv, dtype=np.float32)
    w_final = np.asarray(w_final, dtype=np.float32)
    E = EMBED_DIM

    tri1 = np.where(np.arange(128)[:, None] <= np.arange(128)[None, :], 1.0, 0.0)
    tri = np.concatenate([tri1, tri1], axis=1).astype(bf16)
    in_maps = []
    for c in range(N_CORES):
        b = c // 4
        g = c % 4
        heads = [3 * g, 3 * g + 1, 3 * g + 2]
        hr = [np.arange(64 * h, 64 * h + 64) for h in heads]
        # [q0 q1 | k0 k1 | q2 | k2]
        rows_qk = np.concatenate([hr[0], hr[1], E + hr[0], E + hr[1], hr[2], E + hr[2]])
        rows_v = np.concatenate(hr) + 2 * E
        xT = np.ascontiguousarray(x[b].T).astype(bf16)               # [768, 2048]
        wqk = np.ascontiguousarray(w_qkv[rows_qk].T).astype(bf16)    # [768, 384]
        wv = np.ascontiguousarray(w_qkv[rows_v].T).astype(bf16)      # [768, 192]
        bqk = np.ascontiguousarray(b_qkv[rows_qk][:, None])          # [384, 1] f32
        bvr = np.ascontiguousarray(b_qkv[rows_v][None, :]).astype(bf16)  # [1, 192]
        wf = np.ascontiguousarray(w_final[:, np.concatenate(hr)].T)  # [192, 768]
        wf01 = np.ascontiguousarray(wf[0:128]).astype(bf16)
        wf2 = np.ascontiguousarray(wf[128:192]).astype(bf16)
        in_maps.append({"xT": xT, "wqk": wqk, "wv": wv, "bqk": bqk, "bvr": bvr,
                        "wf01": wf01, "wf2": wf2, "tri": tri})
    return in_maps


def kernel(x, w_qkv, b_qkv, w_final, _trace=False):
    from concourse.bass_utils import run_bass_kernel_spmd

    if "nc" not in _state:
        _state["nc"] = _build()
    nc = _state["nc"]

    in_maps = _prep_inputs(x, w_qkv, b_qkv, w_final)
    res = run_bass_kernel_spmd(nc, in_maps, list(range(N_CORES)), trace=_trace)
    _state["last_result"] = res

    out = np.empty((B, T, EMBED_DIM), dtype=np.float32)
    for b in range(B):
        acc = np.zeros((T, EMBED_DIM), dtype=np.float64)
        for g in range(4):
            acc += res.results[4 * b + g]["out_p"].astype(np.float64)
        out[b] = acc.astype(np.float32)
    return out


# revision 19
# speedup vs baseline: 1.2802x; 1.2802x over previous
"""Multi-head causal attention (B=2, T=2048, E=768, H=12, D=64) on 8 trn2 cores.

Sharding: core c handles batch b=c//4 and heads [3g, 3g+1, 3g+2] (g=c%4).
Each core computes its 3 heads' attention plus their partial contribution to the
final projection; the host sums the 4 partials per batch.

Per-core device program (all matmuls bf16 inputs, fp32 PSUM accumulation):
  phase 1: qT/kT = Wqk^T x (+bias via DVE on the PSUM->SBUF move), laid out
           [q0q1][k0k1][q2|k2] over partitions, bf16 in SBUF.
           v computed directly in [token, d] layout: lhsT = xT tile
           (stationary), rhs = wv chunk; bias via a K=1 ones-row matmul.
           v_all [128, NT, 3, 65] holds [v | 1] per head (ones col set once).
  phase 2: per head h, query quarter q (512 cols): for key blocks j in pairs:
           S^T pair -> one merged PSUM tile [128,2,512]; exp on ACT (merged
           over the pair for full blocks); causal masking applied AFTER exp by
           multiplying the diagonal 128-strips of P with a 0/1 lower-triangle
           (one all-SBUF bf16 DVE op per pair); [O^T; l] accumulated via matmul
           with lhsT=[v_j | 1].  Normalize: DVE reciprocal of l, GPSIMD
           partition_broadcast, DVE multiply -> ot in bf16 SBUF.
  phase 3: out = sum_h O_h^T.T @ wf_h -> [2048, 768] fp32 partial, DMA out.
           Interleaved per query quarter to spread PE work and output DMA.

`repeat` unrolls the whole body N times in one NEFF; used by test.py to
measure per-body HW time as a slope over repeat counts.
"""
import numpy as np

EMBED_DIM = 768
B = 2
T = 2048
N_CORES = 8
NT = T // 128           # 16 token tiles
SCALE = 1.0 / np.sqrt(64.0)

_state = {}


def _build(repeat=1):
    import concourse.tile as tile
    from concourse import bacc, mybir

    F32 = mybir.dt.float32
    BF16 = mybir.dt.bfloat16

    nc = bacc.Bacc("TRN2", target_bir_lowering=False, debug=False)

    xT_d = nc.dram_tensor("xT", [EMBED_DIM, T], BF16, kind="ExternalInput").ap()
    # columns ordered [q0 q1 | k0 k1 | q2 | k2]
    wqk_d = nc.dram_tensor("wqk", [EMBED_DIM, 384], BF16, kind="ExternalInput").ap()
    wv_d = nc.dram_tensor("wv", [EMBED_DIM, 192], BF16, kind="ExternalInput").ap()
    bqk_d = nc.dram_tensor("bqk", [384, 1], F32, kind="ExternalInput").ap()
    bvr_d = nc.dram_tensor("bvr", [1, 192], BF16, kind="ExternalInput").ap()
    wf01_d = nc.dram_tensor("wf01", [128, EMBED_DIM], BF16, kind="ExternalInput").ap()
    wf2_d = nc.dram_tensor("wf2", [64, EMBED_DIM], BF16, kind="ExternalInput").ap()
    tri_d = nc.dram_tensor("tri", [128, 256], BF16, kind="ExternalInput").ap()
    import os
    out_dt = BF16 if os.environ.get("KOUT", "f32") == "bf16" else F32
    out_d = nc.dram_tensor("out_p", [T, EMBED_DIM], out_dt, kind="ExternalOutput").ap()

    with tile.TileContext(nc) as tc:
        with tc.tile_pool(name="const", bufs=1) as const, \
             tc.tile_pool(name="persist", bufs=1) as persist:
            # ---- constants ----
            wqk_sb = const.tile([128, 6, 384], BF16)
            wv_sb = const.tile([128, 6, 192], BF16)
            nc.sync.dma_start(out=wqk_sb[:], in_=wqk_d.rearrange("(k p) c -> p k c", p=128))
            nc.sync.dma_start(out=wv_sb[:], in_=wv_d.rearrange("(k p) c -> p k c", p=128))
            bqk_sb = [const.tile([128, 1], F32, name=f"bqk{m}", tag=f"bqk{m}")
                      for m in range(3)]
            for m in range(3):
                nc.sync.dma_start(out=bqk_sb[m][:], in_=bqk_d[128 * m:128 * (m + 1), :])
            bvr_sb = const.tile([1, 192], BF16)
            nc.sync.dma_start(out=bvr_sb[:], in_=bvr_d[:])
            wf01_sb = const.tile([128, EMBED_DIM], BF16)
            wf2_sb = const.tile([64, EMBED_DIM], BF16)
            nc.sync.dma_start(out=wf01_sb[:], in_=wf01_d[:])
            nc.sync.dma_start(out=wf2_sb[:], in_=wf2_d[:])
            tri2_sb = const.tile([128, 2, 128], BF16)
            nc.sync.dma_start(out=tri2_sb[:], in_=tri_d.rearrange("p (a c) -> p a c", a=2))
            ones_row = const.tile([1, 128], BF16)
            nc.vector.memset(ones_row[:], 1.0)

            # ---- persistent activations ----
            qA = persist.tile([128, T], BF16)   # q0 @0:64, q1 @64:128
            kA = persist.tile([128, T], BF16)   # k0 @0:64, k1 @64:128
            qB = persist.tile([64, T], BF16)    # q2
            kB = persist.tile([64, T], BF16)    # k2
            v_all = persist.tile([128, NT, 3, 65], BF16)   # [v | 1] per head
            nc.vector.memset(v_all[:, :, :, 64:65], 1.0)
            ot01 = persist.tile([128, T], BF16)  # normalized O^T heads 0/1
            ot2 = persist.tile([64, T], BF16)    # head 2

            for rep in range(repeat):
                _emit_body(nc, tc, rep, locals())

    nc.compile()
    return nc


def _emit_body(nc, tc, rep, env):
    """Emit one full forward pass.

    PSUM (8 banks): ps0 ps1 ([128,512] phase-1/v/phase-3 rotation),
    stA stB ([128,2,512] S^T pair tiles), otl0 otl1 ([128,512] O^T+l)."""
    from concourse import mybir

    F32 = mybir.dt.float32
    BF16 = mybir.dt.bfloat16
    Exp = mybir.ActivationFunctionType.Exp
    MULT = mybir.AluOpType.mult

    xT_d, out_d = env["xT_d"], env["out_d"]
    wqk_sb, wv_sb = env["wqk_sb"], env["wv_sb"]
    bqk_sb, bvr_sb = env["bqk_sb"], env["bvr_sb"]
    wf01_sb, wf2_sb = env["wf01_sb"], env["wf2_sb"]
    tri2_sb, ones_row = env["tri2_sb"], env["ones_row"]
    qA, kA, qB, kB = env["qA"], env["kA"], env["qB"], env["kB"]
    v_all = env["v_all"]
    ot01, ot2 = env["ot01"], env["ot2"]

    with tc.tile_pool(name=f"sb{rep}", bufs=1) as sbp, \
         tc.tile_pool(name=f"ps{rep}", bufs=1, space="PSUM") as psp:
        # ---- input DMA: one chunk per query quarter, kicked from SP ----
        xT_sb = sbp.tile([128, 6, T], BF16, name=f"xT{rep}", tag=f"xT{rep % 2}")
        xT_r = xT_d.rearrange("(k p) c -> p k c", p=128)
        for n in range(4):
            nsl = slice(512 * n, 512 * (n + 1))
            nc.sync.dma_start(out=xT_sb[:, :, nsl], in_=xT_r[:, :, nsl])

        pidx = [0]

        def qk_group(m, n):
            # m: 0=[q0q1]->qA, 1=[k0k1]->kA, 2=[q2|k2]->qB(0:64)+kB(64:128)
            c0, c1 = 128 * m, 128 * (m + 1)
            ps = psp.tile([128, 512], F32, name=f"pg{rep}_{m}{n}",
                          tag=f"ps{pidx[0] % 2}")
            pidx[0] += 1
            nsl = slice(512 * n, 512 * (n + 1))
            for k in range(6):
                nc.tensor.matmul(ps[:], lhsT=wqk_sb[:, k, c0:c1],
                                 rhs=xT_sb[:, k, nsl], start=(k == 0), stop=(k == 5))
            with nc.allow_low_precision(reason="bf16 activations"):
                if m < 2:
                    dst = qA if m == 0 else kA
                    nc.vector.tensor_scalar_add(out=dst[:, nsl], in0=ps[:],
                                                scalar1=bqk_sb[m][:])
                else:
                    nc.vector.tensor_scalar_add(out=qB[:, nsl], in0=ps[0:64, :],
                                                scalar1=bqk_sb[2][0:64, :])
                    nc.vector.tensor_scalar_add(out=kB[:, nsl], in0=ps[64:128, :],
                                                scalar1=bqk_sb[2][64:128, :])

        def v_tile(i):
            # v in [token, d] layout for the 3 heads + bias via K=1 matmul
            ps = psp.tile([128, 192], F32, name=f"pv{rep}_{i}",
                          tag=f"ps{pidx[0] % 2}")
            pidx[0] += 1
            tsl = slice(128 * i, 128 * (i + 1))
            for k in range(6):
                nc.tensor.matmul(ps[:], lhsT=xT_sb[:, k, tsl],
                                 rhs=wv_sb[:, k, :], start=(k == 0), stop=False)
            nc.tensor.matmul(ps[:], lhsT=ones_row[:], rhs=bvr_sb[:],
                             start=False, stop=True)
            with nc.allow_low_precision(reason="bf16 activations"):
                nc.vector.tensor_copy(out=v_all[:, i, :, 0:64], in_=ps[:])

        head_cfg = [(qA, kA, 0), (qA, kA, 64), (qB, kB, 0)]
        sidx = [0]
        ptidx = [0]
        rbidx = [0]
        otlidx = [0]

        def attn_quarter(h, q, carry=None):
            """Attention for head h, queries [512q, 512(q+1)).

            Emits all but the last group's PV + the normalize; returns a
            closure that emits them.  The caller invokes it after the next
            quarter's first S groups so PE has work during the trailing exp
            (must be flushed before any phase-3 reader of ot01/ot2).
            """
            qT, kT, o = head_cfg[h]
            base = 512 * q
            ngroups = 2 * q + 2          # key-block pairs (2 items each)
            otl = psp.tile([128, 512], F32, name=f"otl{rep}_{h}{q}",
                           tag=f"otl{otlidx[0] % 2}")
            otlidx[0] += 1
            groups = []
            for g in range(ngroups):
                items = []
                for jj in range(2):
                    j = 2 * g + jj
                    s0 = max(base, 128 * j)
                    items.append((j, s0, base + 512 - s0))
                st = psp.tile([128, 2, 512], F32, name=f"st{rep}_{h}{q}{g}",
                              tag=f"st{sidx[0] % 2}")
                pt = sbp.tile([128, 2, 512], BF16, name=f"pt{rep}_{h}{q}{g}",
                              tag=f"pt{ptidx[0] % 4}")
                sidx[0] += 1
                ptidx[0] += 1
                groups.append((g, items, st, pt))

            def emit_s(g, items, st, pt):
                for jj, (j, s0, ln) in enumerate(items):
                    nc.tensor.matmul(
                        st[:, jj, 0:ln],
                        lhsT=kT[o:o + 64, 128 * j:128 * (j + 1)],
                        rhs=qT[o:o + 64, s0:s0 + ln],
                        start=True, stop=True)

            def emit_exp(g, items, st, pt):
                diag = items[0][1] == 128 * items[0][0]  # s0 == 128j
                with nc.allow_low_precision(reason="bf16 P"):
                    if not diag:
                        nc.scalar.activation(out=pt[:, :, :], in_=st[:, :, :],
                                             func=Exp, scale=float(SCALE))
                    else:
                        for jj, (j, s0, ln) in enumerate(items):
                            nc.scalar.activation(
                                out=pt[:, jj, 0:ln], in_=st[:, jj, 0:ln],
                                func=Exp, scale=float(SCALE))
                        # zero the masked upper triangle of the diagonal
                        # strips (one all-SBUF bf16 DVE op for both items)
                        nc.vector.tensor_tensor(
                            out=pt[:, :, 0:128], in0=pt[:, :, 0:128],
                            in1=tri2_sb[:], op=MULT)

            def emit_pv(g, items, st, pt):
                for jj, (j, s0, ln) in enumerate(items):
                    nc.tensor.matmul(
                        otl[0:65, s0 - base:512],
                        lhsT=v_all[:, j, h, :], rhs=pt[:, jj, 0:ln],
                        start=(j == 0), stop=(j == 2 * ngroups - 1))

            # software-pipelined emission: S g+1 runs on PE while exp g is on
            # ACT; PV g follows its exp.
            emit_s(*groups[0])
            if ngroups > 1:
                emit_s(*groups[1])
            if carry is not None:
                carry()
            emit_exp(*groups[0])
            for g in range(ngroups - 1):
                if g + 2 < ngroups:
                    emit_s(*groups[g + 2])
                emit_exp(*groups[g + 1])
                emit_pv(*groups[g])

            def finish():
                emit_pv(*groups[ngroups - 1])
                # normalize: O^T rows / l row (broadcast the reciprocal over
                # partitions, either on GPSIMD or via a PE ones-matmul)
                import os
                knorm = os.environ.get("KNORM", "pool")
                rsb = sbp.tile([1, 512], F32, name=f"rs{rep}_{h}{q}",
                               tag=f"rs{rbidx[0] % 2}")
                bcast = sbp.tile([64, 512], F32, name=f"bc{rep}_{h}{q}",
                                 tag=f"bc{rbidx[0] % 2}")
                rbidx[0] += 1
                ot_dst = ot2 if h == 2 else ot01
                ob_ = 64 if h == 1 else 0
                with nc.allow_low_precision(reason="recip + bf16 O"):
                    if knorm == "pool":
                        nc.vector.reciprocal(out=rsb[:], in_=otl[64:65, :])
                        nc.gpsimd.partition_broadcast(bcast[:], rsb[:])
                    else:
                        rsb_h = sbp.tile([1, 512], BF16, name=f"rh{rep}_{h}{q}",
                                         tag=f"rh{(rbidx[0] + 1) % 2}")
                        nc.vector.reciprocal(out=rsb_h[:], in_=otl[64:65, :])
                        bcp = psp.tile([64, 512], F32, name=f"bcp{rep}_{h}{q}",
                                       tag=f"ps{pidx[0] % 2}")
                        pidx[0] += 1
                        nc.tensor.matmul(bcp[:], lhsT=ones_row[0:1, 0:64],
                                         rhs=rsb_h[:], start=True, stop=True)
                        nc.vector.tensor_copy(out=bcast[:], in_=bcp[:])
                    nc.vector.tensor_tensor(
                        out=ot_dst[ob_:ob_ + 64, base:base + 512],
                        in0=otl[0:64, :], in1=bcast[:], op=MULT)
            return finish

        def phase3_tile(i):
            a = pidx[0] % 2
            fpa = psp.tile([128, 512], F32, name=f"fpa{rep}_{i}",
                           tag=f"ps{a}")
            fpb = psp.tile([128, 256], F32, name=f"fpb{rep}_{i}",
                           tag=f"ps{1 - a}")
            pidx[0] += 1
            tsl = slice(128 * i, 128 * (i + 1))
            # consecutive matmuls share lhsT (one LDWEIGHTS per operand)
            for (fp, n0, n1) in [(fpa, 0, 512), (fpb, 512, 768)]:
                nc.tensor.matmul(fp[:, 0:n1 - n0], lhsT=ot01[:, tsl],
                                 rhs=wf01_sb[:, n0:n1], start=True, stop=False)
            for (fp, n0, n1) in [(fpa, 0, 512), (fpb, 512, 768)]:
                nc.tensor.matmul(fp[:, 0:n1 - n0], lhsT=ot2[:, tsl],
                                 rhs=wf2_sb[:, n0:n1], start=False, stop=True)
            ob = sbp.tile([128, EMBED_DIM], out_d.dtype, name=f"ob{rep}_{i}",
                          tag=f"ob{i % 3}")
            with nc.allow_low_precision(reason="partial out"):
                nc.vector.tensor_copy(out=ob[:, 0:512], in_=fpa[:])
            nc.vector.tensor_copy(out=ob[:, 512:768], in_=fpb[:])
            nc.sync.dma_start(out=out_d[tsl, :], in_=ob[:])

        # ---- staged interleaved emission ----
        # KPHASES env: "1" = projections only, "12" = + attention, default all
        import os
        kp = os.environ.get("KPHASES", "123")
        if "2" not in kp:
            def attn_quarter(h, q, carry=None):
                if carry is not None:
                    carry()
        if "3" not in kp:
            def phase3_tile(i):
                pass

        carry = [None]

        def attn(h, q):
            carry[0] = attn_quarter(h, q, carry[0])

        def flush():
            if carry[0] is not None:
                carry[0]()
                carry[0] = None

        for n in (0, 1):
            qk_group(0, n)
            qk_group(1, n)
            qk_group(2, n)
            for i in range(4 * n, 4 * n + 4):
                v_tile(i)
        for h in range(3):
            attn(h, 0)
        qk_group(0, 2)
        qk_group(1, 2)
        qk_group(2, 2)
        for i in range(8, 12):
            v_tile(i)
        flush()
        phase3_tile(0)
        phase3_tile(1)
        for h in range(3):
            attn(h, 1)
        qk_group(0, 3)
        qk_group(1, 3)
        qk_group(2, 3)
        for i in range(12, 16):
            v_tile(i)
        flush()
        phase3_tile(2)
        phase3_tile(3)
        for h in range(3):
            attn(h, 2)
        flush()
        for i in range(4, 8):
            phase3_tile(i)
        for h in range(3):
            attn(h, 3)
        flush()
        for i in range(8, 16):
            phase3_tile(i)


def _prep_inputs(x, w_qkv, b_qkv, w_final):
    """Build the 8 per-core input maps from the full inputs."""
    import ml_dtypes
    bf16 = ml_dtypes.bfloat16

    x = np.asarray(x, dtype=np.float32)
    w_qkv = np.asarray(w_qkv, dtype=np.float32)
    b_qkv = np.asarray(b_qkv, dtype=np.float32)
    w_final = np.asarray(w_final, dtype=np.float32)
    E = EMBED_DIM

    tri1 = np.where(np.arange(128)[:, None] <= np.arange(128)[None, :], 1.0, 0.0)
    tri = np.concatenate([tri1, tri1], axis=1).astype(bf16)
    in_maps = []
    for c in range(N_CORES):
        b = c // 4
        g = c % 4
        heads = [3 * g, 3 * g + 1, 3 * g + 2]
        hr = [np.arange(64 * h, 64 * h + 64) for h in heads]
        # [q0 q1 | k0 k1 | q2 | k2]
        rows_qk = np.concatenate([hr[0], hr[1], E + hr[0], E + hr[1], hr[2], E + hr[2]])
        rows_v = np.concatenate(hr) + 2 * E
        xT = np.ascontiguousarray(x[b].T).astype(bf16)               # [768, 2048]
        wqk = np.ascontiguousarray(w_qkv[rows_qk].T).astype(bf16)    # [768, 384]
        wv = np.ascontiguousarray(w_qkv[rows_v].T).astype(bf16)      # [768, 192]
        bqk = np.ascontiguousarray(b_qkv[rows_qk][:, None])          # [384, 1] f32
        bvr = np.ascontiguousarray(b_qkv[rows_v][None, :]).astype(bf16)  # [1, 192]
        wf = np.ascontiguousarray(w_final[:, np.concatenate(hr)].T)  # [192, 768]
        wf01 = np.ascontiguousarray(wf[0:128]).astype(bf16)
        wf2 = np.ascontiguousarray(wf[128:192]).astype(bf16)
        in_maps.append({"xT": xT, "wqk": wqk, "wv": wv, "bqk": bqk, "bvr": bvr,
                        "wf01": wf01, "wf2": wf2, "tri": tri})
    return in_maps


def kernel(x, w_qkv, b_qkv, w_final, _trace=False):
    from concourse.bass_utils import run_bass_kernel_spmd

    if "nc" not in _state:
        _state["nc"] = _build()
    nc = _state["nc"]

    in_maps = _prep_inputs(x, w_qkv, b_qkv, w_final)
    res = run_bass_kernel_spmd(nc, in_maps, list(range(N_CORES)), trace=_trace)
    _state["last_result"] = res

    out = np.empty((B, T, EMBED_DIM), dtype=np.float32)
    for b in range(B):
        acc = np.zeros((T, EMBED_DIM), dtype=np.float64)
        for g in range(4):
            acc += res.results[4 * b + g]["out_p"].astype(np.float64)
        out[b] = acc.astype(np.float32)
    return out
